# revision 11
# baseline (speedup 1.0000x reference)
"""Spatial self-attention (SAGAN-style) kernel for 8 Trainium2 NeuronCores.

Math (per batch b):
    xf  = x[b].reshape(C, N)                       # C=256, N=4096
    qT  = wq @ xf + bq                             # [32, N]
    kT  = wk @ xf                                  # [32, N]  (bk dropped: a
                                                   #  per-query constant shift
                                                   #  of E cancels in softmax)
    V0  = g*wv @ xf                                # [C, N]   (g*bv folded into
                                                   #  the residual on host)
    E^T = kT.T @ qT                                # [m, n]  (keys on partitions)
    A'  = exp(E^T)          (no max-subtraction: |E| < 29, safe in fp32)
    s   = colsum(A')                               # softmax denominator
    out = g*(V0 @ A / s) + (x + g*bv)

Sharding: core i handles batch b = i//2, query half h = i%2 (2048 queries).
Each core computes kT / V^T for the full 4096 keys of its batch. The host
rotates xf per-core so the core's 2048 query columns always sit at columns
0..2047 (attention reductions are permutation-invariant over keys).

Device layout (O^T formulation, 256-query chunks):
  - wq/wk shipped 4x column-replicated [C,128] so the projection matmul
    emits qT/kT with the d-dim already replicated across all four 32-row
    groups (no SBUF replication DMAs).
  - E^T via 4-way row-packed K=32 matmuls (tile_position=(32r,0)), one
    [128,1024] PSUM pair-of-banks per group of 4 key blocks; ONE exp
    ACTIVATE per group.
  - O^T = A'.T @ [gV^T | 1] accumulated in two [128,257] PSUM tiles per
    256-query chunk; the ones column yields the softmax denominator free.
  - per-chunk input tiles + DMAs split across the SP and Activation HWDGE
    queues so the first projection matmul starts ~1.5us in.
  - PSUM budget: tag "e" 2x[128,1024] (4 banks) + tags "ot0"/"ot1"
    2x[128,257] each (4 banks) = 8 banks; phase-1 projection PSUMs reuse
    the same slots.
  - phase-1 PSUM extraction split between ACT (pure copies) and DVE
    (qT bias add) so neither engine gates the projections.
  - walrus allows at most ONE semaphore wait per TPB instruction; Tile's
    surplus waits are legalized post-hoc (_strip_self_waits,
    _split_multi_waits).
"""

import ml_dtypes
import numpy as np

import concourse.bass as bass
import concourse.mybir as mybir
import concourse.tile as tile
from concourse.bass import ts
from concourse.bass_utils import run_bass_kernel_spmd

B, C, HH, WW = 4, 256, 64, 64
N = HH * WW          # 4096 spatial positions
D = 32               # C // 8 head dim
NCORES = 8
NQ = N * B // NCORES  # 2048 queries per core
MB = N // 128        # 32 key blocks
NCH = NQ // 256      # 8 query chunks of 256 per core
NG = N // 512        # 8 key groups of 4 blocks
NSL = NQ // 128      # 16 query slices of 128

F32 = mybir.dt.float32
BF16 = mybir.dt.bfloat16
AF = mybir.ActivationFunctionType
OP = mybir.AluOpType

VW = C + 2          # vT block width: 256 channels + ones col + pad


def _build():
    nc = bass.Bass()
    xfb = nc.declare_dram_parameter("xfb", [C, N], BF16, isOutput=False)
    xtq = nc.declare_dram_parameter("xtq", [NQ, C], BF16, isOutput=False)
    wq4 = nc.declare_dram_parameter("wq4", [C, 128], BF16, isOutput=False)
    wk4 = nc.declare_dram_parameter("wk4", [C, 128], BF16, isOutput=False)
    wvT = nc.declare_dram_parameter("wvT", [C, C], BF16, isOutput=False)
    bq4 = nc.declare_dram_parameter("bq4", [128, 1], F32, isOutput=False)
    outT = nc.declare_dram_parameter("outT", [NQ, C], BF16, isOutput=True)

    with tile.TileContext(nc) as tc:
        with (
            tc.tile_pool(name="const", bufs=1) as constp,
            tc.tile_pool(name="xfp", bufs=1) as xfp,
            tc.tile_pool(name="big", bufs=1) as bigp,
            tc.tile_pool(name="apool", bufs=6) as apool,
            tc.tile_pool(name="fin", bufs=2) as finp,
            tc.tile_pool(name="ps", bufs=1, space="PSUM") as psp,
        ):
            wq_t = [constp.tile([128, 128], BF16, name=f"wq{i}") for i in range(2)]
            wk_t = [constp.tile([128, 128], BF16, name=f"wk{i}") for i in range(2)]
            wv_t = [constp.tile([128, C], BF16, name=f"wv{i}") for i in range(2)]
            bq_t = constp.tile([128, 1], F32, name="bq")
            xf_t = [[xfp.tile([128, 512], BF16, name=f"xf{i}_{c}")
                     for c in range(NG)] for i in range(2)]
            kT_t = [bigp.tile([128, 512], BF16, name=f"kT{c}") for c in range(NG)]
            qT_t = [bigp.tile([128, 512], BF16, name=f"qT{c}") for c in range(4)]
            vT_t = [bigp.tile([128, 4 * VW], BF16, name=f"vT{g}") for g in range(NG)]
            xt = bigp.tile([128, NSL * C], BF16, name="xt")

            # --- input DMAs: weights first, xf chunks split across the two
            # HWDGE queues (SP + Activation), residual x^T last ---
            nc.sync.dma_start(wq_t[0][:], wq4[0:128, :])
            nc.sync.dma_start(wq_t[1][:], wq4[128:256, :])
            nc.sync.dma_start(wk_t[0][:], wk4[0:128, :])
            nc.sync.dma_start(wk_t[1][:], wk4[128:256, :])
            nc.sync.dma_start(bq_t[:], bq4[:, :])
            nc.sync.dma_start(wv_t[0][:], wvT[0:128, :])
            nc.sync.dma_start(wv_t[1][:], wvT[128:256, :])
            for c in range(NG):
                for i in range(2):
                    eng = nc.sync if i == 0 else nc.scalar
                    eng.dma_start(xf_t[i][c][:],
                                  xfb[i * 128:(i + 1) * 128, ts(c, 512)])
            for g in range(NG):
                nc.vector.memset(vT_t[g][:, C::VW], 1.0)

            # PE warm-up: dense zero matmuls while input DMAs stream in, so
            # HAM unthrottles (K=8/8) before phase 1 issues real work
            wz = constp.tile([128, 512], BF16, name="wz")
            nc.vector.memset(wz[:], 0.0)
            pswu = psp.tile([128, 2048], F32, tag="e", name="pswu")
            for w in range(10):
                nc.tensor.matmul(pswu[:, ts(w % 4, 512)], lhsT=wz[:, 0:128],
                                 rhs=wz[:], start=True, stop=True,
                                 skip_group_check=True)

            # --- phase 1a: qT/kT projections. PSUMs live in column regions
            # of the 4-bank "e" slot (3 generations of 4) so the ot tags stay
            # free for the main loop and the e slot frees early. kT copies
            # alternate ACT/DVE; qT bias-adds on DVE. ---
            p1e = [psp.tile([128, 2048], F32, tag="e", name=f"p1e{i}")
                   for i in range(3)]
            kq_slots = []
            for c in range(NG):
                kq_slots.append(("k", c))
                if c < 4:
                    kq_slots.append(("q", c))
            for i, (kind, c) in enumerate(kq_slots):
                ps = p1e[i // 4][:, ts(i % 4, 512)]
                w_t = wk_t if kind == "k" else wq_t
                for cb in range(2):
                    nc.tensor.matmul(
                        ps, lhsT=w_t[cb][:], rhs=xf_t[cb][c][:],
                        start=(cb == 0), stop=(cb == 1), skip_group_check=True)
                if kind == "k":
                    if c % 2 == 0:
                        nc.scalar.copy(kT_t[c][:], ps)
                    else:
                        nc.vector.tensor_copy(kT_t[c][:], ps)
                else:
                    nc.vector.tensor_scalar_add(qT_t[c][:], ps, bq_t[:, 0:1])

            # --- phase 1b: V^T blocks through the ot ring, extraction on DVE
            # only (ACT starts the exp chain concurrently) ---
            for c in range(NG):
                for r in range(4):
                    m = 4 * c + r
                    psv = psp.tile([128, C], F32, tag=f"ot{m % 4}",
                                   name=f"psv{m}")
                    for cb in range(2):
                        nc.tensor.matmul(
                            psv[:], lhsT=xf_t[cb][c][:, ts(r, 128)],
                            rhs=wv_t[cb][:],
                            start=(cb == 0), stop=(cb == 1),
                            skip_group_check=True)
                    nc.vector.tensor_copy(vT_t[c][:, r * VW:r * VW + C],
                                          psv[:])

            # residual x^T DMA emitted late: keeps it out of the DMA-sem
            # thresholds that gate phase-1 matmuls (needed only at chunk fin)
            nc.scalar.dma_start(
                xt[:].rearrange("p (s c) -> p s c", c=C),
                xtq.rearrange("(s p) c -> p s c", p=128))

            # --- phase 2: units u = (cp, g) over 512-query chunk-pairs.
            # e spans 4 PSUM banks; the 4 row-packed E matmuls each own one
            # bank (concurrent single-port writes to one bank are illegal).
            # Software-pipelined by one unit: emit E(u), exp(u), then the
            # O matmuls of unit u-1 so the PE streams O(u-1) while ACT runs
            # exp(u) and never idles behind the single-buffered e tile. ---
            NU = (NCH // 2) * NG
            ot_cur = None
            pend = None     # (ot tiles, a tile, g, cp) of unit u-1

            def emit_o(p):
                pot, pa, pg, pcp = p
                for r in range(4):
                    m = 4 * pg + r
                    st, sp = (m == 0), (m == MB - 1)
                    for j in range(4):
                        nc.tensor.matmul(
                            pot[j][:],
                            lhsT=pa[:, 512 * r + 128 * j:
                                    512 * r + 128 * (j + 1)],
                            rhs=vT_t[pg][:, r * VW:r * VW + C + 1],
                            start=st, stop=sp, skip_group_check=True)
                if pg == NG - 1:
                    for j in range(4):
                        sl = 4 * pcp + j
                        rcp = finp.tile([128, 1], F32, tag="r", bufs=4,
                                        name=f"r{pcp}_{j}")
                        nc.vector.reciprocal(rcp[:], pot[j][:, C:C + 1])
                        t = finp.tile([128, C], F32, tag="t", bufs=4,
                                      name=f"t{pcp}_{j}")
                        if j % 2 == 0:
                            nc.scalar.activation(t[:], pot[j][:, 0:C],
                                                 AF.Copy, scale=rcp[:, 0:1])
                        else:
                            nc.vector.tensor_scalar_mul(t[:], pot[j][:, 0:C],
                                                        rcp[:, 0:1])
                        f = finp.tile([128, C], BF16, tag="f", bufs=4,
                                      name=f"f{pcp}_{j}")
                        nc.vector.tensor_tensor(f[:], t[:], xt[:, ts(sl, C)],
                                                OP.add)
                        nc.sync.dma_start(outT[sl * 128:(sl + 1) * 128, :],
                                          f[:])

            for u in range(NU):
                cp, g = u // NG, u % NG
                if g == 0:
                    ot_cur = [psp.tile([128, C + 1], F32, tag=f"ot{j}",
                                       name=f"ot{j}_{cp}") for j in range(4)]
                e = psp.tile([128, 2048], F32, tag="e", name=f"e{cp}_{g}")
                for r in range(4):
                    nc.tensor.matmul(
                        e[:, ts(r, 512)],
                        lhsT=kT_t[g][32 * r:32 * (r + 1), ts(r, 128)],
                        rhs=qT_t[cp][32 * r:32 * (r + 1), :],
                        start=True, stop=True, skip_group_check=True,
                        tile_position=(32 * r, 0),
                    )
                a = apool.tile([128, 2048], BF16, tag="a", name=f"a{cp}_{g}")
                nc.scalar.activation(a[:], e[:], AF.Exp)
                if pend is not None:
                    emit_o(pend)
                pend = (ot_cur, a, g, cp)
            emit_o(pend)
    _strip_self_waits(nc)
    _split_multi_waits(nc)
    return nc


_ENGINE_SEM_PREFIX = {
    "EngineType.PE": "PE_",
    "EngineType.DVE": "DVE_",
    "EngineType.Activation": "Activation_",
    "EngineType.Pool": "Pool_",
    "EngineType.SP": "SP_",
}


def _strip_self_waits(nc):
    """Drop same-engine semaphore waits from multi-wait TPB instructions.

    Walrus allows exactly one sync wait per TPB instruction. Tile emits
    redundant self-engine waits (WAW on pool-slot reuse, RAW from same-engine
    producers): each engine executes its queue in order, so a wait on the
    engine's own semaphore is always satisfied by program order. Dropping
    them collapses every instruction to at most one (cross-engine) wait.
    """
    for bb in nc.m.functions[0].blocks:
        for inst in bb.instructions:
            si = inst.sync_info
            if si is None:
                continue
            w = si.on_wait
            if len(w) <= 1 or inst.opcode == "Drain":
                continue
            pfx = _ENGINE_SEM_PREFIX.get(str(inst.engine))
            if pfx is None:
                continue
            kept = [x for x in w if not x.ant_name.startswith(pfx)]
            if kept and len(kept) < len(w):
                si.on_wait = kept


def _split_multi_waits(nc):
    """Walrus allows one sync wait per TPB instruction; move surplus waits
    onto dedicated single-wait Drain instructions inserted just before the
    offender (same engine, executes in order)."""
    import bass_rust
    cnt = 0
    for bb in nc.m.functions[0].blocks:
        il = bb.instructions
        i = 0
        while i < len(il):
            inst = il[i]
            si = inst.sync_info
            w = si.on_wait if si else []
            if len(w) > 1:
                for j, wait in enumerate(w[:-1]):
                    d = mybir.InstDrain(name=f"{inst.name}-w{j}", ins=[], outs=[],
                                        bass_is_fusable=False)
                    d.engine = inst.engine
                    d.sync_info = bass_rust.SyncInfo(on_wait=[wait], on_update=[])
                    il.insert(i, d)
                    i += 1
                    cnt += 1
                si.on_wait = [w[-1]]
            i += 1
    return cnt


_NC_CACHE = None


def _get_nc():
    global _NC_CACHE
    if _NC_CACHE is None:
        _NC_CACHE = _build()
    return _NC_CACHE


def kernel(x, wq, bq, wk, bk, wv, bv, gamma, _trace=False):
    f32 = lambda a: np.ascontiguousarray(np.asarray(a, dtype=np.float32))
    bf16 = lambda a: np.ascontiguousarray(np.asarray(a, dtype=np.float32)
                                          .astype(ml_dtypes.bfloat16))
    x = f32(x)
    g = float(np.asarray(gamma).reshape(-1)[0])
    xfull = x.reshape(B, C, N)
    shared = {
        "wq4": bf16(np.tile(np.asarray(wq).T, (1, 4))),
        "wk4": bf16(np.tile(np.asarray(wk).T, (1, 4))),
        "wvT": bf16((g * np.asarray(wv)).T),
        "bq4": f32(np.tile(np.asarray(bq).reshape(D, 1), (128 // D, 1))),
    }
    gbv_row = (g * np.asarray(bv, dtype=np.float32)).reshape(1, C)
    in_maps = []
    for core in range(NCORES):
        b, h = core // 2, core % 2
        m = dict(shared)
        if h == 0:
            xr = xfull[b]
        else:
            # rotate so this core's query half sits at columns 0..NQ-1;
            # key order is irrelevant (attention reduces over all keys)
            xr = np.concatenate([xfull[b][:, NQ:], xfull[b][:, :NQ]], axis=1)
        m["xfb"] = bf16(xr)
        m["xtq"] = bf16(xr[:, :NQ].T + gbv_row)
        in_maps.append(m)

    res = run_bass_kernel_spmd(_get_nc(), in_maps, list(range(NCORES)),
                               trace=_trace)
    full = np.empty((B, C, N), np.float32)
    for core in range(NCORES):
        b, h = core // 2, core % 2
        full[b][:, h * NQ:(h + 1) * NQ] = np.asarray(res.results[core]["outT"], dtype=np.float32).T
    out = full.reshape(B, C, HH, WW)
    if _trace:
        return out, res
    return out


# revision 12
# speedup vs baseline: 1.0552x; 1.0552x over previous
"""Spatial self-attention (SAGAN-style) kernel for 8 Trainium2 NeuronCores.

Math (per batch b):
    xf  = x[b].reshape(C, N)                       # C=256, N=4096
    qT  = wq @ xf + bq                             # [32, N]
    kT  = wk @ xf                                  # [32, N]  (bk dropped: a
                                                   #  per-query constant shift
                                                   #  of E cancels in softmax)
    V0  = g*wv @ xf                                # [C, N]   (g*bv folded into
                                                   #  the residual on host)
    E^T = kT.T @ qT                                # [m, n]  (keys on partitions)
    A'  = exp(E^T)          (no max-subtraction: |E| < 29, safe in fp32)
    s   = colsum(A')                               # softmax denominator
    out = g*(V0 @ A / s) + (x + g*bv)

Sharding: core i handles batch b = i//2, query half h = i%2 (2048 queries).
Each core computes kT / V^T for the full 4096 keys of its batch. The host
rotates xf per-core so the core's 2048 query columns always sit at columns
0..2047 (attention reductions are permutation-invariant over keys).

Device layout (O^T formulation, 256-query chunks):
  - wq/wk shipped 4x column-replicated [C,128] so the projection matmul
    emits qT/kT with the d-dim already replicated across all four 32-row
    groups (no SBUF replication DMAs).
  - E^T via 4-way row-packed K=32 matmuls (tile_position=(32r,0)), one
    [128,1024] PSUM pair-of-banks per group of 4 key blocks; ONE exp
    ACTIVATE per group.
  - O^T = A'.T @ [gV^T | 1] accumulated in two [128,257] PSUM tiles per
    256-query chunk; the ones column yields the softmax denominator free.
  - per-chunk input tiles + DMAs split across the SP and Activation HWDGE
    queues so the first projection matmul starts ~1.5us in.
  - PSUM budget: tag "e" 2x[128,1024] (4 banks) + tags "ot0"/"ot1"
    2x[128,257] each (4 banks) = 8 banks; phase-1 projection PSUMs reuse
    the same slots.
  - phase-1 PSUM extraction split between ACT (pure copies) and DVE
    (qT bias add) so neither engine gates the projections.
  - walrus allows at most ONE semaphore wait per TPB instruction; Tile's
    surplus waits are legalized post-hoc (_strip_self_waits,
    _split_multi_waits).
"""

import ml_dtypes
import numpy as np

import concourse.bass as bass
import concourse.mybir as mybir
import concourse.tile as tile
from concourse.bass import ts
from concourse.bass_utils import run_bass_kernel_spmd

B, C, HH, WW = 4, 256, 64, 64
N = HH * WW          # 4096 spatial positions
D = 32               # C // 8 head dim
NCORES = 8
NQ = N * B // NCORES  # 2048 queries per core
MB = N // 128        # 32 key blocks
NCH = NQ // 256      # 8 query chunks of 256 per core
NG = N // 512        # 8 key groups of 4 blocks
NSL = NQ // 128      # 16 query slices of 128

F32 = mybir.dt.float32
BF16 = mybir.dt.bfloat16
AF = mybir.ActivationFunctionType
OP = mybir.AluOpType

VW = C + 2          # vT block width: 256 channels + ones col + pad


def _build():
    nc = bass.Bass()
    xfb = nc.declare_dram_parameter("xfb", [C, N], BF16, isOutput=False)
    xtq = nc.declare_dram_parameter("xtq", [NQ, C], BF16, isOutput=False)
    wq4 = nc.declare_dram_parameter("wq4", [C, 128], BF16, isOutput=False)
    wk4 = nc.declare_dram_parameter("wk4", [C, 128], BF16, isOutput=False)
    wvT = nc.declare_dram_parameter("wvT", [C, C], BF16, isOutput=False)
    bq4 = nc.declare_dram_parameter("bq4", [128, 1], F32, isOutput=False)
    outT = nc.declare_dram_parameter("outT", [NQ, C], BF16, isOutput=True)

    with tile.TileContext(nc) as tc:
        with (
            tc.tile_pool(name="const", bufs=1) as constp,
            tc.tile_pool(name="xfp", bufs=1) as xfp,
            tc.tile_pool(name="big", bufs=1) as bigp,
            tc.tile_pool(name="apool", bufs=6) as apool,
            tc.tile_pool(name="fin", bufs=2) as finp,
            tc.tile_pool(name="ps", bufs=1, space="PSUM") as psp,
        ):
            wq_t = [constp.tile([128, 128], BF16, name=f"wq{i}") for i in range(2)]
            wk_t = [constp.tile([128, 128], BF16, name=f"wk{i}") for i in range(2)]
            wv_t = [constp.tile([128, C], BF16, name=f"wv{i}") for i in range(2)]
            bq_t = constp.tile([128, 1], F32, name="bq")
            xf_t = [[xfp.tile([128, 512], BF16, name=f"xf{i}_{c}")
                     for c in range(NG)] for i in range(2)]
            kT_t = [bigp.tile([128, 512], BF16, name=f"kT{c}") for c in range(NG)]
            qT_t = [bigp.tile([128, 512], BF16, name=f"qT{c}") for c in range(4)]
            vT_t = [bigp.tile([128, 4 * VW], BF16, name=f"vT{g}") for g in range(NG)]
            xt = bigp.tile([128, NSL * C], BF16, name="xt")

            # --- input DMAs: weights first, xf chunks split across the two
            # HWDGE queues (SP + Activation), residual x^T last ---
            nc.sync.dma_start(wq_t[0][:], wq4[0:128, :])
            nc.sync.dma_start(wq_t[1][:], wq4[128:256, :])
            nc.sync.dma_start(wk_t[0][:], wk4[0:128, :])
            nc.sync.dma_start(wk_t[1][:], wk4[128:256, :])
            nc.sync.dma_start(bq_t[:], bq4[:, :])
            nc.sync.dma_start(wv_t[0][:], wvT[0:128, :])
            nc.sync.dma_start(wv_t[1][:], wvT[128:256, :])
            for c in range(NG):
                for i in range(2):
                    eng = nc.sync if i == 0 else nc.scalar
                    eng.dma_start(xf_t[i][c][:],
                                  xfb[i * 128:(i + 1) * 128, ts(c, 512)])
            for g in range(NG):
                nc.vector.memset(vT_t[g][:, C::VW], 1.0)

            # PE warm-up: dense zero matmuls while input DMAs stream in, so
            # HAM unthrottles (K=8/8) before phase 1 issues real work
            wz = constp.tile([128, 512], BF16, name="wz")
            nc.vector.memset(wz[:], 0.0)
            pswu = psp.tile([128, 2048], F32, tag="e", name="pswu")
            for w in range(10):
                nc.tensor.matmul(pswu[:, ts(w % 4, 512)], lhsT=wz[:, 0:128],
                                 rhs=wz[:], start=True, stop=True,
                                 skip_group_check=True)

            # phase-1 PSUM ring: 5 slots (ot0..ot3 = 1 bank each, e = 4 banks)
            _ring = ["ot0", "ot1", "ot2", "ot3", "e"]
            _rix = [0]

            def p1tile(shape, nm):
                t = psp.tile(shape, F32, tag=_ring[_rix[0] % 5], name=nm)
                _rix[0] += 1
                return t

            # --- phase 1: per key chunk c: qT/kT projections then V^T blocks,
            # pipelined right behind the arriving xf DMAs. Extraction load
            # balanced across ACT and DVE. ---
            for c in range(NG):
                psk = p1tile([128, 512], f"psk{c}")
                for cb in range(2):
                    nc.tensor.matmul(
                        psk[:], lhsT=wk_t[cb][:], rhs=xf_t[cb][c][:],
                        start=(cb == 0), stop=(cb == 1), skip_group_check=True)
                if c % 2 == 0:
                    nc.scalar.copy(kT_t[c][:], psk[:])
                else:
                    nc.vector.tensor_copy(kT_t[c][:], psk[:])
                if c < 4:
                    psq = p1tile([128, 512], f"psq{c}")
                    for cb in range(2):
                        nc.tensor.matmul(
                            psq[:], lhsT=wq_t[cb][:], rhs=xf_t[cb][c][:],
                            start=(cb == 0), stop=(cb == 1),
                            skip_group_check=True)
                    nc.vector.tensor_scalar_add(qT_t[c][:], psq[:],
                                                bq_t[:, 0:1])
                for r in range(4):
                    m = 4 * c + r
                    psv = p1tile([128, C], f"psv{m}")
                    for cb in range(2):
                        nc.tensor.matmul(
                            psv[:], lhsT=xf_t[cb][c][:, ts(r, 128)],
                            rhs=wv_t[cb][:],
                            start=(cb == 0), stop=(cb == 1),
                            skip_group_check=True)
                    dst = vT_t[c][:, r * VW:r * VW + C]
                    if m % 2 == 0:
                        nc.scalar.copy(dst, psv[:])
                    else:
                        nc.vector.tensor_copy(dst, psv[:])

            # residual x^T DMA emitted late: keeps it out of the DMA-sem
            # thresholds that gate phase-1 matmuls (needed only at chunk fin)
            nc.scalar.dma_start(
                xt[:].rearrange("p (s c) -> p s c", c=C),
                xtq.rearrange("(s p) c -> p s c", p=128))

            # --- phase 2: units u = (cp, g) over 512-query chunk-pairs.
            # e spans 4 PSUM banks; the 4 row-packed E matmuls each own one
            # bank (concurrent single-port writes to one bank are illegal).
            # Software-pipelined by one unit: emit E(u), exp(u), then the
            # O matmuls of unit u-1 so the PE streams O(u-1) while ACT runs
            # exp(u) and never idles behind the single-buffered e tile. ---
            NU = (NCH // 2) * NG
            ot_cur = None
            pend = None     # (ot tiles, a tile, g, cp) of unit u-1

            def emit_o(p):
                pot, pa, pg, pcp = p
                for r in range(4):
                    m = 4 * pg + r
                    st, sp = (m == 0), (m == MB - 1)
                    for j in range(4):
                        nc.tensor.matmul(
                            pot[j][:],
                            lhsT=pa[:, 512 * r + 128 * j:
                                    512 * r + 128 * (j + 1)],
                            rhs=vT_t[pg][:, r * VW:r * VW + C + 1],
                            start=st, stop=sp, skip_group_check=True)
                if pg == NG - 1:
                    for j in range(4):
                        sl = 4 * pcp + j
                        rcp = finp.tile([128, 1], F32, tag="r", bufs=4,
                                        name=f"r{pcp}_{j}")
                        nc.vector.reciprocal(rcp[:], pot[j][:, C:C + 1])
                        t = finp.tile([128, C], F32, tag="t", bufs=4,
                                      name=f"t{pcp}_{j}")
                        if j % 2 == 0:
                            nc.scalar.activation(t[:], pot[j][:, 0:C],
                                                 AF.Copy, scale=rcp[:, 0:1])
                        else:
                            nc.vector.tensor_scalar_mul(t[:], pot[j][:, 0:C],
                                                        rcp[:, 0:1])
                        f = finp.tile([128, C], BF16, tag="f", bufs=4,
                                      name=f"f{pcp}_{j}")
                        nc.vector.tensor_tensor(f[:], t[:], xt[:, ts(sl, C)],
                                                OP.add)
                        nc.sync.dma_start(outT[sl * 128:(sl + 1) * 128, :],
                                          f[:])

            for u in range(NU):
                cp, g = u // NG, u % NG
                if g == 0:
                    ot_cur = [psp.tile([128, C + 1], F32, tag=f"ot{j}",
                                       name=f"ot{j}_{cp}") for j in range(4)]
                e = psp.tile([128, 2048], F32, tag="e", name=f"e{cp}_{g}")
                for r in range(4):
                    nc.tensor.matmul(
                        e[:, ts(r, 512)],
                        lhsT=kT_t[g][32 * r:32 * (r + 1), ts(r, 128)],
                        rhs=qT_t[cp][32 * r:32 * (r + 1), :],
                        start=True, stop=True, skip_group_check=True,
                        tile_position=(32 * r, 0),
                    )
                a = apool.tile([128, 2048], BF16, tag="a", name=f"a{cp}_{g}")
                nc.scalar.activation(a[:], e[:], AF.Exp)
                if pend is not None:
                    emit_o(pend)
                pend = (ot_cur, a, g, cp)
            emit_o(pend)
    _strip_self_waits(nc)
    _split_multi_waits(nc)
    return nc


_ENGINE_SEM_PREFIX = {
    "EngineType.PE": "PE_",
    "EngineType.DVE": "DVE_",
    "EngineType.Activation": "Activation_",
    "EngineType.Pool": "Pool_",
    "EngineType.SP": "SP_",
}


def _strip_self_waits(nc):
    """Drop same-engine semaphore waits from multi-wait TPB instructions.

    Walrus allows exactly one sync wait per TPB instruction. Tile emits
    redundant self-engine waits (WAW on pool-slot reuse, RAW from same-engine
    producers): each engine executes its queue in order, so a wait on the
    engine's own semaphore is always satisfied by program order. Dropping
    them collapses every instruction to at most one (cross-engine) wait.
    """
    for bb in nc.m.functions[0].blocks:
        for inst in bb.instructions:
            si = inst.sync_info
            if si is None:
                continue
            w = si.on_wait
            if len(w) <= 1 or inst.opcode == "Drain":
                continue
            pfx = _ENGINE_SEM_PREFIX.get(str(inst.engine))
            if pfx is None:
                continue
            kept = [x for x in w if not x.ant_name.startswith(pfx)]
            if kept and len(kept) < len(w):
                si.on_wait = kept


def _split_multi_waits(nc):
    """Walrus allows one sync wait per TPB instruction; move surplus waits
    onto dedicated single-wait Drain instructions inserted just before the
    offender (same engine, executes in order)."""
    import bass_rust
    cnt = 0
    for bb in nc.m.functions[0].blocks:
        il = bb.instructions
        i = 0
        while i < len(il):
            inst = il[i]
            si = inst.sync_info
            w = si.on_wait if si else []
            if len(w) > 1:
                for j, wait in enumerate(w[:-1]):
                    d = mybir.InstDrain(name=f"{inst.name}-w{j}", ins=[], outs=[],
                                        bass_is_fusable=False)
                    d.engine = inst.engine
                    d.sync_info = bass_rust.SyncInfo(on_wait=[wait], on_update=[])
                    il.insert(i, d)
                    i += 1
                    cnt += 1
                si.on_wait = [w[-1]]
            i += 1
    return cnt


_NC_CACHE = None


def _get_nc():
    global _NC_CACHE
    if _NC_CACHE is None:
        _NC_CACHE = _build()
    return _NC_CACHE


def kernel(x, wq, bq, wk, bk, wv, bv, gamma, _trace=False):
    f32 = lambda a: np.ascontiguousarray(np.asarray(a, dtype=np.float32))
    bf16 = lambda a: np.ascontiguousarray(np.asarray(a, dtype=np.float32)
                                          .astype(ml_dtypes.bfloat16))
    x = f32(x)
    g = float(np.asarray(gamma).reshape(-1)[0])
    xfull = x.reshape(B, C, N)
    shared = {
        "wq4": bf16(np.tile(np.asarray(wq).T, (1, 4))),
        "wk4": bf16(np.tile(np.asarray(wk).T, (1, 4))),
        "wvT": bf16((g * np.asarray(wv)).T),
        "bq4": f32(np.tile(np.asarray(bq).reshape(D, 1), (128 // D, 1))),
    }
    gbv_row = (g * np.asarray(bv, dtype=np.float32)).reshape(1, C)
    in_maps = []
    for core in range(NCORES):
        b, h = core // 2, core % 2
        m = dict(shared)
        if h == 0:
            xr = xfull[b]
        else:
            # rotate so this core's query half sits at columns 0..NQ-1;
            # key order is irrelevant (attention reduces over all keys)
            xr = np.concatenate([xfull[b][:, NQ:], xfull[b][:, :NQ]], axis=1)
        m["xfb"] = bf16(xr)
        m["xtq"] = bf16(xr[:, :NQ].T + gbv_row)
        in_maps.append(m)

    res = run_bass_kernel_spmd(_get_nc(), in_maps, list(range(NCORES)),
                               trace=_trace)
    full = np.empty((B, C, N), np.float32)
    for core in range(NCORES):
        b, h = core // 2, core % 2
        full[b][:, h * NQ:(h + 1) * NQ] = np.asarray(res.results[core]["outT"], dtype=np.float32).T
    out = full.reshape(B, C, HH, WW)
    if _trace:
        return out, res
    return out


# revision 13
# speedup vs baseline: 1.0705x; 1.0145x over previous
"""Spatial self-attention (SAGAN-style) kernel for 8 Trainium2 NeuronCores.

Math (per batch b):
    xf  = x[b].reshape(C, N)                       # C=256, N=4096
    qT  = wq @ xf + bq                             # [32, N]
    kT  = wk @ xf                                  # [32, N]  (bk dropped: a
                                                   #  per-query constant shift
                                                   #  of E cancels in softmax)
    V0  = g*wv @ xf                                # [C, N]   (g*bv folded into
                                                   #  the residual on host)
    E^T = kT.T @ qT                                # [m, n]  (keys on partitions)
    A'  = exp(E^T)          (no max-subtraction: |E| < 29, safe in fp32)
    s   = colsum(A')                               # softmax denominator
    out = g*(V0 @ A / s) + (x + g*bv)

Sharding: core i handles batch b = i//2, query half h = i%2 (2048 queries).
Each core computes kT / V^T for the full 4096 keys of its batch. The host
rotates xf per-core so the core's 2048 query columns always sit at columns
0..2047 (attention reductions are permutation-invariant over keys).

Device layout (O^T formulation, 256-query chunks):
  - wq/wk shipped 4x column-replicated [C,128] so the projection matmul
    emits qT/kT with the d-dim already replicated across all four 32-row
    groups (no SBUF replication DMAs).
  - E^T via 4-way row-packed K=32 matmuls (tile_position=(32r,0)), one
    [128,1024] PSUM pair-of-banks per group of 4 key blocks; ONE exp
    ACTIVATE per group.
  - O^T = A'.T @ [gV^T | 1] accumulated in two [128,257] PSUM tiles per
    256-query chunk; the ones column yields the softmax denominator free.
  - per-chunk input tiles + DMAs split across the SP and Activation HWDGE
    queues so the first projection matmul starts ~1.5us in.
  - PSUM budget: tag "e" 2x[128,1024] (4 banks) + tags "ot0"/"ot1"
    2x[128,257] each (4 banks) = 8 banks; phase-1 projection PSUMs reuse
    the same slots.
  - phase-1 PSUM extraction split between ACT (pure copies) and DVE
    (qT bias add) so neither engine gates the projections.
  - walrus allows at most ONE semaphore wait per TPB instruction; Tile's
    surplus waits are legalized post-hoc (_strip_self_waits,
    _split_multi_waits).
"""

import ml_dtypes
import numpy as np

import concourse.bass as bass
import concourse.mybir as mybir
import concourse.tile as tile
from concourse.bass import ts
from concourse.bass_utils import run_bass_kernel_spmd

B, C, HH, WW = 4, 256, 64, 64
N = HH * WW          # 4096 spatial positions
D = 32               # C // 8 head dim
NCORES = 8
NQ = N * B // NCORES  # 2048 queries per core
MB = N // 128        # 32 key blocks
NCH = NQ // 256      # 8 query chunks of 256 per core
NG = N // 512        # 8 key groups of 4 blocks
NSL = NQ // 128      # 16 query slices of 128

F32 = mybir.dt.float32
BF16 = mybir.dt.bfloat16
AF = mybir.ActivationFunctionType
OP = mybir.AluOpType

VW = C + 2          # vT block width: 256 channels + ones col + pad


def _build():
    nc = bass.Bass()
    xfb = nc.declare_dram_parameter("xfb", [C, N], BF16, isOutput=False)
    xtq = nc.declare_dram_parameter("xtq", [NQ, C], BF16, isOutput=False)
    wq4 = nc.declare_dram_parameter("wq4", [C, 128], BF16, isOutput=False)
    wk4 = nc.declare_dram_parameter("wk4", [C, 128], BF16, isOutput=False)
    wvT = nc.declare_dram_parameter("wvT", [C, C], BF16, isOutput=False)
    bq4 = nc.declare_dram_parameter("bq4", [128, 1], F32, isOutput=False)
    outT = nc.declare_dram_parameter("outT", [NQ, C], BF16, isOutput=True)

    with tile.TileContext(nc) as tc:
        with (
            tc.tile_pool(name="const", bufs=1) as constp,
            tc.tile_pool(name="xfp", bufs=1) as xfp,
            tc.tile_pool(name="big", bufs=1) as bigp,
            tc.tile_pool(name="apool", bufs=6) as apool,
            tc.tile_pool(name="fin", bufs=2) as finp,
            tc.tile_pool(name="ps", bufs=1, space="PSUM") as psp,
        ):
            wq_t = [constp.tile([128, 128], BF16, name=f"wq{i}") for i in range(2)]
            wk_t = [constp.tile([128, 128], BF16, name=f"wk{i}") for i in range(2)]
            wv_t = [constp.tile([128, C], BF16, name=f"wv{i}") for i in range(2)]
            bq_t = constp.tile([128, 1], F32, name="bq")
            xf_t = [[xfp.tile([128, 512], BF16, name=f"xf{i}_{c}")
                     for c in range(NG)] for i in range(2)]
            kT_t = [bigp.tile([128, 512], BF16, name=f"kT{c}") for c in range(NG)]
            qT_t = [bigp.tile([128, 512], BF16, name=f"qT{c}") for c in range(4)]
            vT_t = [bigp.tile([128, 4 * VW], BF16, name=f"vT{g}") for g in range(NG)]
            xt = bigp.tile([128, NSL * C], BF16, name="xt")

            # --- input DMAs: weights first, xf chunks split across the two
            # HWDGE queues (SP + Activation), residual x^T last ---
            nc.sync.dma_start(wq_t[0][:], wq4[0:128, :])
            nc.sync.dma_start(wq_t[1][:], wq4[128:256, :])
            nc.sync.dma_start(wk_t[0][:], wk4[0:128, :])
            nc.sync.dma_start(wk_t[1][:], wk4[128:256, :])
            nc.sync.dma_start(bq_t[:], bq4[:, :])
            nc.sync.dma_start(wv_t[0][:], wvT[0:128, :])
            nc.sync.dma_start(wv_t[1][:], wvT[128:256, :])
            for g in range(NG):
                nc.vector.memset(vT_t[g][:, C::VW], 1.0)

            # PE warm-up: dense zero matmuls while input DMAs stream in, so
            # HAM unthrottles (K=8/8) before phase 1 issues real work
            wz = constp.tile([128, 512], BF16, name="wz")
            nc.vector.memset(wz[:], 0.0)
            pswu = psp.tile([128, 2048], F32, tag="e", name="pswu")
            for w in range(10):
                nc.tensor.matmul(pswu[:, ts(w % 4, 512)], lhsT=wz[:, 0:128],
                                 rhs=wz[:], start=True, stop=True,
                                 skip_group_check=True)

            # phase-1 PSUM ring: 5 slots (ot0..ot3 = 1 bank each, e = 4 banks)
            _ring = ["ot0", "ot1", "ot2", "ot3", "e"]
            _rix = [0]

            def p1tile(shape, nm):
                t = psp.tile(shape, F32, tag=_ring[_rix[0] % 5], name=nm)
                _rix[0] += 1
                return t

            # --- phase 1: per key chunk c: DMA the chunk just-in-time (so
            # Tile's schedule-position DMA-sem thresholds stay minimal), then
            # qT/kT projections and V^T blocks. Extraction load balanced
            # across ACT and DVE. ---
            for c in range(NG):
                for i in range(2):
                    nc.sync.dma_start(xf_t[i][c][:],
                                      xfb[i * 128:(i + 1) * 128, ts(c, 512)])
                psk = p1tile([128, 512], f"psk{c}")
                for cb in range(2):
                    nc.tensor.matmul(
                        psk[:], lhsT=wk_t[cb][:], rhs=xf_t[cb][c][:],
                        start=(cb == 0), stop=(cb == 1), skip_group_check=True)
                if c % 2 == 0:
                    nc.scalar.copy(kT_t[c][:], psk[:])
                else:
                    nc.vector.tensor_copy(kT_t[c][:], psk[:])
                if c < 4:
                    psq = p1tile([128, 512], f"psq{c}")
                    for cb in range(2):
                        nc.tensor.matmul(
                            psq[:], lhsT=wq_t[cb][:], rhs=xf_t[cb][c][:],
                            start=(cb == 0), stop=(cb == 1),
                            skip_group_check=True)
                    nc.vector.tensor_scalar_add(qT_t[c][:], psq[:],
                                                bq_t[:, 0:1])
                for r in range(4):
                    m = 4 * c + r
                    psv = p1tile([128, C], f"psv{m}")
                    for cb in range(2):
                        nc.tensor.matmul(
                            psv[:], lhsT=xf_t[cb][c][:, ts(r, 128)],
                            rhs=wv_t[cb][:],
                            start=(cb == 0), stop=(cb == 1),
                            skip_group_check=True)
                    dst = vT_t[c][:, r * VW:r * VW + C]
                    if m % 2 == 0:
                        nc.scalar.copy(dst, psv[:])
                    else:
                        nc.vector.tensor_copy(dst, psv[:])

            # residual x^T DMA emitted late: keeps it out of the DMA-sem
            # thresholds that gate phase-1 matmuls (needed only at chunk fin)
            nc.scalar.dma_start(
                xt[:].rearrange("p (s c) -> p s c", c=C),
                xtq.rearrange("(s p) c -> p s c", p=128))

            # --- phase 2: units u = (cp, g) over 512-query chunk-pairs.
            # e spans 4 PSUM banks; the 4 row-packed E matmuls each own one
            # bank (concurrent single-port writes to one bank are illegal).
            # Software-pipelined by one unit: emit E(u), exp(u), then the
            # O matmuls of unit u-1 so the PE streams O(u-1) while ACT runs
            # exp(u) and never idles behind the single-buffered e tile. ---
            NU = (NCH // 2) * NG
            ot_cur = None
            pend = None     # (ot tiles, a tile, g, cp) of unit u-1

            def emit_o(p):
                pot, pa, pg, pcp = p
                for r in range(4):
                    m = 4 * pg + r
                    st, sp = (m == 0), (m == MB - 1)
                    for j in range(4):
                        nc.tensor.matmul(
                            pot[j][:],
                            lhsT=pa[:, 512 * r + 128 * j:
                                    512 * r + 128 * (j + 1)],
                            rhs=vT_t[pg][:, r * VW:r * VW + C + 1],
                            start=st, stop=sp, skip_group_check=True)
                if pg == NG - 1:
                    for j in range(4):
                        sl = 4 * pcp + j
                        rcp = finp.tile([128, 1], F32, tag="r", bufs=4,
                                        name=f"r{pcp}_{j}")
                        nc.vector.reciprocal(rcp[:], pot[j][:, C:C + 1])
                        t = finp.tile([128, C], F32, tag="t", bufs=4,
                                      name=f"t{pcp}_{j}")
                        if j % 2 == 0:
                            nc.scalar.activation(t[:], pot[j][:, 0:C],
                                                 AF.Copy, scale=rcp[:, 0:1])
                        else:
                            nc.vector.tensor_scalar_mul(t[:], pot[j][:, 0:C],
                                                        rcp[:, 0:1])
                        f = finp.tile([128, C], BF16, tag="f", bufs=4,
                                      name=f"f{pcp}_{j}")
                        nc.vector.tensor_tensor(f[:], t[:], xt[:, ts(sl, C)],
                                                OP.add)
                        nc.sync.dma_start(outT[sl * 128:(sl + 1) * 128, :],
                                          f[:])

            for u in range(NU):
                cp, g = u // NG, u % NG
                if g == 0:
                    ot_cur = [psp.tile([128, C + 1], F32, tag=f"ot{j}",
                                       name=f"ot{j}_{cp}") for j in range(4)]
                e = psp.tile([128, 2048], F32, tag="e", name=f"e{cp}_{g}")
                for r in range(4):
                    nc.tensor.matmul(
                        e[:, ts(r, 512)],
                        lhsT=kT_t[g][32 * r:32 * (r + 1), ts(r, 128)],
                        rhs=qT_t[cp][32 * r:32 * (r + 1), :],
                        start=True, stop=True, skip_group_check=True,
                        tile_position=(32 * r, 0),
                    )
                a = apool.tile([128, 2048], BF16, tag="a", name=f"a{cp}_{g}")
                nc.scalar.activation(a[:], e[:], AF.Exp)
                if pend is not None:
                    emit_o(pend)
                pend = (ot_cur, a, g, cp)
            emit_o(pend)
    _strip_self_waits(nc)
    _split_multi_waits(nc)
    return nc


_ENGINE_SEM_PREFIX = {
    "EngineType.PE": "PE_",
    "EngineType.DVE": "DVE_",
    "EngineType.Activation": "Activation_",
    "EngineType.Pool": "Pool_",
    "EngineType.SP": "SP_",
}


def _strip_self_waits(nc):
    """Drop same-engine semaphore waits from multi-wait TPB instructions.

    Walrus allows exactly one sync wait per TPB instruction. Tile emits
    redundant self-engine waits (WAW on pool-slot reuse, RAW from same-engine
    producers): each engine executes its queue in order, so a wait on the
    engine's own semaphore is always satisfied by program order. Dropping
    them collapses every instruction to at most one (cross-engine) wait.
    """
    for bb in nc.m.functions[0].blocks:
        for inst in bb.instructions:
            si = inst.sync_info
            if si is None:
                continue
            w = si.on_wait
            if len(w) <= 1 or inst.opcode == "Drain":
                continue
            pfx = _ENGINE_SEM_PREFIX.get(str(inst.engine))
            if pfx is None:
                continue
            kept = [x for x in w if not x.ant_name.startswith(pfx)]
            if kept and len(kept) < len(w):
                si.on_wait = kept


def _split_multi_waits(nc):
    """Walrus allows one sync wait per TPB instruction; move surplus waits
    onto dedicated single-wait Drain instructions inserted just before the
    offender (same engine, executes in order)."""
    import bass_rust
    cnt = 0
    for bb in nc.m.functions[0].blocks:
        il = bb.instructions
        i = 0
        while i < len(il):
            inst = il[i]
            si = inst.sync_info
            w = si.on_wait if si else []
            if len(w) > 1:
                for j, wait in enumerate(w[:-1]):
                    d = mybir.InstDrain(name=f"{inst.name}-w{j}", ins=[], outs=[],
                                        bass_is_fusable=False)
                    d.engine = inst.engine
                    d.sync_info = bass_rust.SyncInfo(on_wait=[wait], on_update=[])
                    il.insert(i, d)
                    i += 1
                    cnt += 1
                si.on_wait = [w[-1]]
            i += 1
    return cnt


_NC_CACHE = None


def _get_nc():
    global _NC_CACHE
    if _NC_CACHE is None:
        _NC_CACHE = _build()
    return _NC_CACHE


def kernel(x, wq, bq, wk, bk, wv, bv, gamma, _trace=False):
    f32 = lambda a: np.ascontiguousarray(np.asarray(a, dtype=np.float32))
    bf16 = lambda a: np.ascontiguousarray(np.asarray(a, dtype=np.float32)
                                          .astype(ml_dtypes.bfloat16))
    x = f32(x)
    g = float(np.asarray(gamma).reshape(-1)[0])
    xfull = x.reshape(B, C, N)
    shared = {
        "wq4": bf16(np.tile(np.asarray(wq).T, (1, 4))),
        "wk4": bf16(np.tile(np.asarray(wk).T, (1, 4))),
        "wvT": bf16((g * np.asarray(wv)).T),
        "bq4": f32(np.tile(np.asarray(bq).reshape(D, 1), (128 // D, 1))),
    }
    gbv_row = (g * np.asarray(bv, dtype=np.float32)).reshape(1, C)
    in_maps = []
    for core in range(NCORES):
        b, h = core // 2, core % 2
        m = dict(shared)
        if h == 0:
            xr = xfull[b]
        else:
            # rotate so this core's query half sits at columns 0..NQ-1;
            # key order is irrelevant (attention reduces over all keys)
            xr = np.concatenate([xfull[b][:, NQ:], xfull[b][:, :NQ]], axis=1)
        m["xfb"] = bf16(xr)
        m["xtq"] = bf16(xr[:, :NQ].T + gbv_row)
        in_maps.append(m)

    res = run_bass_kernel_spmd(_get_nc(), in_maps, list(range(NCORES)),
                               trace=_trace)
    full = np.empty((B, C, N), np.float32)
    for core in range(NCORES):
        b, h = core // 2, core % 2
        full[b][:, h * NQ:(h + 1) * NQ] = np.asarray(res.results[core]["outT"], dtype=np.float32).T
    out = full.reshape(B, C, HH, WW)
    if _trace:
        return out, res
    return out
